# revision 3
# baseline (speedup 1.0000x reference)
"""MHSA Trainium2 kernel v2: B=2, N=2048, H=1024, 16 heads x d=64.

Sharding: 8 cores = 2 (batch) x 4 (head-groups of 4 heads), fully
independent; host concatenates per-core [2048, 256] outputs.

Per-core plan (all matmul inputs bf16, PSUM f32):
  - QK projection -> QT2/KT2 [d, tok] per head-pair (head parity picks
    partition half), scores computed transposed: S^T[j, i] per j-tile.
  - exp via ACT (scale fused), P -> SBUF bf16 [j, 2jt*512i] tiles.
  - attnV with P as the STATIONARY operand: out[i-chunk 128, 65] = sum_j
    P[j, i]^T @ Vau[j, 65]; Vau has V plus a mask column so row 64
    accumulates the softmax denominator; moving size is 65 so the PE cost
    halves vs the [65, i] orientation, and the output lands in natural
    [token, dim] orientation with the denominator as a per-partition
    scalar (normalize = reciprocal + tensor_scalar_mul, no broadcast).
  - V projection and the h2/h3 QK projection are woven into the
    attention stream as PE filler (ACT is the critical engine there).
"""

import numpy as np

import concourse.bass as bass
import concourse.bacc as bacc
import concourse.mybir as mybir
import concourse.tile as tile
from concourse.bass_utils import run_bass_kernel_spmd

F32 = mybir.dt.float32
F32R = mybir.dt.float32r
BF16 = mybir.dt.bfloat16
AF = mybir.ActivationFunctionType

HID = 1024
NT = 2048
D = 64
HPC = 4  # heads per core
NCORES = 8
SCALE = float(HID) ** -0.5
KD = HID // 128  # 8 contraction chunks
NJT = NT // 128  # 16 j-tiles
IB = 512  # i-block
NIB = NT // IB  # 4

_CACHE = {}


def _build():
    if "nc" in _CACHE:
        return _CACHE["nc"]
    nc = bacc.Bacc("TRN2", debug=False)
    hsT_d = nc.dram_tensor("hsT", [HID, NT], BF16, kind="ExternalInput")
    wqk_d = nc.dram_tensor("wqk", [HID, 8 * D], BF16, kind="ExternalInput")
    wv_d = nc.dram_tensor("wv", [HID, HPC * D], BF16, kind="ExternalInput")
    msk_d = nc.dram_tensor("msk", [NT], F32, kind="ExternalInput")
    out_d = nc.dram_tensor("out", [NT, HPC * D], F32, kind="ExternalOutput")

    with tile.TileContext(nc) as tc:
        with tc.tile_pool(name="per", bufs=1) as per:
            QT2 = [per.tile([128, NT], BF16, tag=f"qt{p}", name=f"qt{p}") for p in range(2)]
            KT2 = [per.tile([128, NT], BF16, tag=f"kt{p}", name=f"kt{p}") for p in range(2)]
            Vau = [per.tile([128, HPC, D + 1], BF16, tag=f"vau{t}", name=f"vau{t}") for t in range(NJT)]
            msk_t = per.tile([128, NJT], F32, tag="msk")
            for t in range(NJT):
                nc.gpsimd.memset(Vau[t][:, :, D : D + 1], 1.0)

            with tc.tile_pool(name="ld", bufs=1) as ld:
                hsT = [ld.tile([128, NT], BF16, tag=f"hst{k}", name=f"hst{k}") for k in range(KD)]
                wqk = [ld.tile([128, 8 * D], BF16, tag=f"wqk{k}", name=f"wqk{k}") for k in range(KD)]
                wv = [ld.tile([128, HPC * D], BF16, tag=f"wv{k}", name=f"wv{k}") for k in range(KD)]
                hsT_r = hsT_d.ap().rearrange("(n p) m -> n p m", p=128)
                wqk_r = wqk_d.ap().rearrange("(n p) m -> n p m", p=128)
                wv_r = wv_d.ap().rearrange("(n p) m -> n p m", p=128)
                for k in range(KD):
                    nc.sync.dma_start(out=wqk[k][:], in_=wqk_r[k])
                    if k < KD - 1:
                        nc.sync.dma_start(out=hsT[k][:], in_=hsT_r[k])
                    else:
                        # split the last chunk: the c0a/c2a slices of the final
                        # k-batch need only tokens 0:1024, so the first scores
                        # can start one half-transfer earlier
                        nc.sync.dma_start(out=hsT[k][:, 0:1024], in_=hsT_r[k][:, 0:1024])
                        nc.sync.dma_start(out=hsT[k][:, 1024:2048], in_=hsT_r[k][:, 1024:2048])
                for k in range(KD):
                    nc.sync.dma_start(out=wv[k][:], in_=wv_r[k])
                nc.sync.dma_start(out=msk_t[:], in_=msk_d.ap().rearrange("(a p) -> p a", p=128))

                # ---- phase A: QK projection for heads 0/1 (c=0 -> q0|q1,
                # c=2 -> k0|k1), k-outer so PE tracks the DMA stream.
                with tc.tile_pool(name="pjA", bufs=1, space="PSUM") as pjA:
                    # slot order fixes PSUM bank order; phase B reuses these
                    # banks as sc <- (c0a, c2a), vo <- c0b, aux <- c2b, so the
                    # copies that gate phase B's first work retire earliest.
                    slots = [
                        (pjA.tile([128, 1024], F32, tag=f"pj{i}", name=f"pj{i}"), c, th)
                        for i, (c, th) in enumerate([(0, 0), (2, 0), (0, 1), (2, 1)])
                    ]
                    # copies are emitted right after each slot's final k-chunk
                    # matmul so they overlap the remaining k7 matmuls; ACT is
                    # idle here so it takes the two K copies.
                    def _qkcopy(i):
                        sl, c, th = slots[i]
                        if c == 0:
                            nc.vector.tensor_copy(QT2[0][:, th * 1024 : (th + 1) * 1024], sl[:])
                        else:
                            nc.scalar.activation(
                                KT2[0][:, th * 1024 : (th + 1) * 1024], sl[:], AF.Copy
                            )

                    for k in range(KD):
                        for i, (sl, c, th) in enumerate(slots):
                            for t2 in range(2):
                                nc.tensor.matmul(
                                    sl[:, t2 * 512 : (t2 + 1) * 512],
                                    wqk[k][:, c * 128 : (c + 1) * 128],
                                    hsT[k][:, th * 1024 + t2 * 512 : th * 1024 + (t2 + 1) * 512],
                                    start=(k == 0),
                                    stop=(k == KD - 1),
                                )
                            if k == KD - 1:
                                _qkcopy(i)

                # ---- phase B: attention with V proj + h2/h3 QK proj woven in
                with (
                    tc.tile_pool(name="psB", bufs=1, space="PSUM") as ps,
                    tc.tile_pool(name="ptp", bufs=1) as ptp,
                    tc.tile_pool(name="stg", bufs=1) as stg,
                ):
                    # filler units: half-granularity so a single unit does not
                    # starve the ACT exp stream (emitted between dependent
                    # attention stages as PE filler).
                    fstate = {}

                    def vproj_half(t, half):
                        if half == 0:
                            fstate[("v", t)] = ps.tile(
                                [128, 512], F32, tag="aux", bufs=1, name=f"pv{t}"
                            )
                        sl = fstate[("v", t)]
                        for k in range(half * 4, half * 4 + 4):
                            nc.tensor.matmul(
                                sl[:, 0:256],
                                hsT[k][:, t * 128 : (t + 1) * 128],
                                wv[k][:],
                                start=(k == 0),
                                stop=(k == KD - 1),
                            )
                        if half == 1:
                            nc.vector.tensor_copy(
                                Vau[t][:, :, 0:D],
                                sl[:, 0:256].rearrange("p (h d) -> p h d", h=HPC),
                            )
                            nc.gpsimd.tensor_scalar_mul(
                                Vau[t][:], Vau[t][:], msk_t[:, t : t + 1]
                            )
                            del fstate[("v", t)]

                    def cproj_half(c, tq, half):
                        if half == 0:
                            fstate[(c, tq)] = ps.tile(
                                [128, 512], F32, tag="aux", bufs=1, name=f"pc{c}_{tq}"
                            )
                        sl = fstate[(c, tq)]
                        for k in range(half * 4, half * 4 + 4):
                            nc.tensor.matmul(
                                sl[:],
                                wqk[k][:, c * 128 : (c + 1) * 128],
                                hsT[k][:, tq * 512 : (tq + 1) * 512],
                                start=(k == 0),
                                stop=(k == KD - 1),
                            )
                        if half == 1:
                            dst = QT2[1] if c == 1 else KT2[1]
                            nc.vector.tensor_copy(dst[:, tq * 512 : (tq + 1) * 512], sl[:])
                            del fstate[(c, tq)]

                    from collections import deque

                    vfill = deque(
                        lambda t=t, hf=hf: vproj_half(t, hf)
                        for t in range(NJT)
                        for hf in (0, 1)
                    )
                    cfill = deque(
                        lambda c=c, tq=tq, hf=hf: cproj_half(c, tq, hf)
                        for tq in range(4)
                        for c in (1, 3)
                        for hf in (0, 1)
                    )

                    NCH = IB // 128  # i-chunks per block
                    # j-tile groups per block: 4 triples + 2 pairs so the sc
                    # slot is [128, 1536] (3 banks x 2 bufs) and most exps run
                    # at ap=1536, amortizing the ACT per-instruction init.
                    GROUPS = [(0, 3), (3, 6), (6, 9), (9, 12), (12, 14), (14, 16)]
                    GSTART = [s for s, _ in GROUPS]

                    def _pslice(Ps, jt, ic):
                        g = max(i for i, s in enumerate(GSTART) if s <= jt)
                        loc = jt - GSTART[g]
                        return Ps[g][:, loc * 512 + ic * 128 : loc * 512 + (ic + 1) * 128]

                    def attnv_piece(nc, vob, Ps, h, ic):
                        """One i-chunk accumulation group: 16 consecutive
                        matmuls (interleaved groups within one PSUM bank are
                        illegal; distinct groups in the same bank must be
                        strictly sequential in PE issue order, which ic-outer
                        emission guarantees)."""
                        for jt in range(NJT):
                            nc.tensor.matmul(
                                vob[:, ic, :],
                                _pslice(Ps, jt, ic),
                                Vau[jt][:, h, :],
                                start=(jt == 0),
                                stop=(jt == NJT - 1),
                            )

                    def store_block(nc, blk, vob, h, i0):
                        ot = stg.tile([128, NCH, D], F32, tag="ot", bufs=2, name=f"ot{blk}")
                        for ic in range(NCH):
                            rcp = stg.tile([128, 1], F32R, tag="rcp", bufs=8, name=f"rcp{blk}_{ic}")
                            with nc.allow_low_precision("f32r is bit-identical to f32"):
                                nc.vector.reciprocal(rcp[:], vob[:, ic, D : D + 1].bitcast(F32R))
                            nc.vector.tensor_scalar_mul(
                                ot[:, ic, :], vob[:, ic, 0:D], rcp[:, 0:1].bitcast(F32)
                            )
                        nc.sync.dma_start(
                            out=out_d.ap()[i0 : i0 + IB, h * D : (h + 1) * D].rearrange(
                                "(ic p) d -> p ic d", p=128
                            ),
                            in_=ot[:],
                        )

                    NBLK = HPC * NIB
                    NG = len(GROUPS)
                    DEFER = 3  # attnV runs this many blocks behind its scores
                    pending = []  # (blk, Ps, h, i0) awaiting attnV, oldest first
                    state = {}  # blk -> vob (allocated at burst emission)
                    for blk in range(NBLK):
                        h, ib = blk // NIB, blk % NIB
                        hp, hoff = h // 2, (h % 2) * 64
                        i0 = ib * IB
                        Ps = []
                        for g, (js, je) in enumerate(GROUPS):
                            gw = (je - js) * 512
                            ssl = ps.tile([128, 1536], F32, tag="sc", bufs=2, name=f"sc{blk}_{g}")
                            for jj in range(je - js):
                                jt = js + jj
                                nc.tensor.matmul(
                                    ssl[:, jj * 512 : (jj + 1) * 512],
                                    KT2[hp][hoff : hoff + 64, jt * 128 : (jt + 1) * 128],
                                    QT2[hp][hoff : hoff + 64, i0 : i0 + IB],
                                    start=True,
                                    stop=True,
                                    tile_position=(hoff, 0),
                                )
                            P = ptp.tile([128, 1536], BF16, tag="pt", bufs=NG * (DEFER + 1),
                                         name=f"pt{blk}_{g}")
                            nc.scalar.activation(P[:, 0:gw], ssl[:, 0:gw], AF.Exp,
                                                 bias=0.0, scale=SCALE)
                            Ps.append(P)
                            # attnV for the block finished DEFER blocks ago, one
                            # i-chunk piece per group slot (keeps PE fed without
                            # head-of-line blocking); vob allocated here so
                            # only one is ever live.
                            if len(pending) >= DEFER and 1 <= g <= NCH + 1:
                                pblk, pPs, ph, pi0 = pending[0]
                                if g == 1:
                                    state[pblk] = ps.tile(
                                        [128, NCH, D + 1], F32, tag="vo", bufs=1, name=f"vo{pblk}"
                                    )
                                if g <= NCH:
                                    attnv_piece(nc, state[pblk], pPs, ph, g - 1)
                                else:
                                    store_block(nc, pblk, state.pop(pblk), ph, pi0)
                                    pending.pop(0)
                            # filler PE work between dependent stages. All V
                            # halves MUST be emitted before the first attnv
                            # piece that reads Vau (else PE head-of-line
                            # deadlocks on a Vau copy whose matmuls sit behind
                            # it in the queue): 2 per group slot drains the 32
                            # halves by block 2 group 3.
                            if vfill:
                                vfill.popleft()()
                                if vfill:
                                    vfill.popleft()()
                            elif cfill and g in (0, 3, 5):
                                cfill.popleft()()
                        pending.append((blk, Ps, h, i0))

                    # epilogue: all but the last pending block run as normal
                    # bursts (their exps retired long ago); the last block's
                    # four i-chunk groups go to FOUR different PSUM banks so
                    # their matmuls can interleave jt-by-jt and retire while
                    # the final exps are still streaming.
                    for pblk, pPs, ph, pi0 in pending[:-1]:
                        vob = ps.tile([128, NCH, D + 1], F32, tag="vo", bufs=1, name=f"vo{pblk}")
                        for ic in range(NCH):
                            attnv_piece(nc, vob, pPs, ph, ic)
                        store_block(nc, pblk, vob, ph, pi0)
                    lblk, lPs, lh, li0 = pending[-1]
                    voA = ps.tile([128, NCH, D + 1], F32, tag="vo", bufs=1, name="voLa")
                    auxA = ps.tile([128, 512], F32, tag="aux", bufs=1, name="auxLa")
                    # one sc slot provides two more banks (cols 0:512 and
                    # 512:1024 live in different banks, so their accumulation
                    # groups may interleave)
                    scX = ps.tile([128, 1536], F32, tag="sc", bufs=2, name="scLx")
                    lvo = [
                        voA[:, 0, :],
                        auxA[:, 0 : D + 1],
                        scX[:, 0 : D + 1],
                        scX[:, 512 : 512 + D + 1],
                    ]
                    for jt in range(NJT):
                        for ic in range(NCH):
                            nc.tensor.matmul(
                                lvo[ic],
                                _pslice(lPs, jt, ic),
                                Vau[jt][:, lh, :],
                                start=(jt == 0),
                                stop=(jt == NJT - 1),
                            )
                    ot = stg.tile([128, NCH, D], F32, tag="ot", bufs=2, name="otL")
                    for ic in range(NCH):
                        rcp = stg.tile([128, 1], F32R, tag="rcp", bufs=8, name=f"rcpL{ic}")
                        with nc.allow_low_precision("f32r is bit-identical to f32"):
                            nc.vector.reciprocal(rcp[:], lvo[ic][:, D : D + 1].bitcast(F32R))
                        nc.vector.tensor_scalar_mul(
                            ot[:, ic, :], lvo[ic][:, 0:D], rcp[:, 0:1].bitcast(F32)
                        )
                    nc.sync.dma_start(
                        out=out_d.ap()[li0 : li0 + IB, lh * D : (lh + 1) * D].rearrange(
                            "(ic p) d -> p ic d", p=128
                        ),
                        in_=ot[:],
                    )
    if not nc.is_finalized():
        nc.finalize()
    _CACHE["nc"] = nc
    return nc


def kernel(hidden_states, attention_mask, W_qkv):
    import ml_dtypes

    hs = np.asarray(hidden_states, dtype=np.float32)  # [2, 2048, 1024]
    am = np.asarray(attention_mask)  # [2, 2048]
    W = np.asarray(W_qkv, dtype=np.float32)  # [16, 1024, 192]

    nc = _build()
    in_maps = []
    for core in range(NCORES):
        b, hg = core // 4, core % 4
        Wc = W[hg * 4 : hg * 4 + 4]  # [4, 1024, 192]
        q = [Wc[h, :, 0:64] for h in range(4)]
        k = [Wc[h, :, 64:128] for h in range(4)]
        v = [Wc[h, :, 128:192] for h in range(4)]
        in_maps.append(
            {
                "hsT": np.ascontiguousarray(hs[b].T).astype(ml_dtypes.bfloat16),
                "wqk": np.ascontiguousarray(
                    np.concatenate([q[0], q[1], q[2], q[3], k[0], k[1], k[2], k[3]], axis=1)
                ).astype(ml_dtypes.bfloat16),
                "wv": np.ascontiguousarray(np.concatenate(v, axis=1)).astype(ml_dtypes.bfloat16),
                "msk": (am[b] != 0).astype(np.float32),
            }
        )
    res = run_bass_kernel_spmd(nc, in_maps, list(range(NCORES)))
    if res.exec_time_ns is not None:
        print(f"HW exec time: {res.exec_time_ns} ns")
    if res.mean_exec_time_ns is not None:
        print(f"HW exec time (mean across cores): {res.mean_exec_time_ns} ns")
    out = np.empty((2, NT, HID), dtype=np.float32)
    for core in range(NCORES):
        b, hg = core // 4, core % 4
        out[b, :, hg * 256 : (hg + 1) * 256] = res.results[core]["out"]
    return out


def predicted_exec_ns():
    """Device-occupancy estimate for one core (all 8 run the same program
    in parallel)."""
    nc = _build()
    from concourse.timeline_sim import TimelineSim
    return float(TimelineSim(nc, trace=False).simulate())


# revision 6
# speedup vs baseline: 1.0075x; 1.0075x over previous
"""MHSA Trainium2 kernel: B=2, N=2048, H=1024, 16 heads x d=64.

Sharding: 8 cores = 2 (batch) x 4 (head-groups of 4 heads), fully
independent (no collectives); host concatenates per-core [2048, 256]
outputs along the hidden axis.

Per-core plan (all matmul inputs bf16, PSUM f32; the steady state is
Activation-engine-bound at ~1 exp elem/lane/cycle, so everything else is
scheduled to hide under the exp stream):
  - QK projection -> QT2/KT2 [d, tok] per head-pair (head parity picks
    the partition half), k-outer so the PE tracks the input DMA stream.
  - scores transposed: S^T[j, i] per j-tile, emitted in groups of 3
    j-tiles into [128, 1536] PSUM slots (2 bufs) so each ACT exp covers
    ap=1536, amortizing the per-instruction init; exp scale fused.
  - attnV with P as the STATIONARY operand: out[i-chunk 128, 65] = sum_j
    P[j, i]^T @ Vau[j, 65]; Vau carries V plus a mask column so row 64
    accumulates the softmax denominator. Moving size 65 halves the PE
    cost vs the [65, i] orientation, the output lands in natural
    [token, dim] orientation, and the denominator is a per-partition
    scalar (normalize = reciprocal + tensor_scalar_mul, no broadcast).
    Masked keys are handled by scaling Vau rows (incl. the ones column)
    by the mask, which equals -inf score masking after normalization.
  - attnV for block n runs DEFER=3 blocks later as four 16-matmul
    accumulation bursts (interleaved groups within one PSUM bank are
    illegal), woven between score/exp stages; V projection and the
    h2/h3 QK projection fill the remaining PE slack.
  - last block: its four accumulators go to four DIFFERENT banks so the
    bursts interleave jt-by-jt and mostly retire before the final exp.
"""

import numpy as np

import concourse.bass as bass
import concourse.bacc as bacc
import concourse.mybir as mybir
import concourse.tile as tile
from concourse.bass_utils import run_bass_kernel_spmd

F32 = mybir.dt.float32
F32R = mybir.dt.float32r
BF16 = mybir.dt.bfloat16
AF = mybir.ActivationFunctionType

HID = 1024
NT = 2048
D = 64
HPC = 4  # heads per core
NCORES = 8
SCALE = float(HID) ** -0.5
KD = HID // 128  # 8 contraction chunks
NJT = NT // 128  # 16 j-tiles
IB = 512  # i-block
NIB = NT // IB  # 4

_CACHE = {}


def _build():
    if "nc" in _CACHE:
        return _CACHE["nc"]
    nc = bacc.Bacc("TRN2", debug=False)
    hsT_d = nc.dram_tensor("hsT", [HID, NT], BF16, kind="ExternalInput")
    wqk_d = nc.dram_tensor("wqk", [HID, 8 * D], BF16, kind="ExternalInput")
    wv_d = nc.dram_tensor("wv", [HID, HPC * D], BF16, kind="ExternalInput")
    msk_d = nc.dram_tensor("msk", [NT], F32, kind="ExternalInput")
    out_d = nc.dram_tensor("out", [NT, HPC * D], F32, kind="ExternalOutput")

    with tile.TileContext(nc) as tc:
        with tc.tile_pool(name="per", bufs=1) as per:
            QT2 = [per.tile([128, NT], BF16, tag=f"qt{p}", name=f"qt{p}") for p in range(2)]
            KT2 = [per.tile([128, NT], BF16, tag=f"kt{p}", name=f"kt{p}") for p in range(2)]
            Vau = [per.tile([128, HPC, D + 1], BF16, tag=f"vau{t}", name=f"vau{t}") for t in range(NJT)]
            msk_t = per.tile([128, NJT], F32, tag="msk")
            for t in range(NJT):
                nc.gpsimd.memset(Vau[t][:, :, D : D + 1], 1.0)

            with tc.tile_pool(name="ld", bufs=1) as ld:
                hsT = [ld.tile([128, NT], BF16, tag=f"hst{k}", name=f"hst{k}") for k in range(KD)]
                wqk_t = ld.tile([128, KD, 8 * D], BF16, tag="wqk", name="wqk_t")
                wqk = [wqk_t[:, k, :] for k in range(KD)]
                wv = [ld.tile([128, HPC * D], BF16, tag=f"wv{k}", name=f"wv{k}") for k in range(KD)]
                hsT_r = hsT_d.ap().rearrange("(n p) m -> n p m", p=128)
                wqk_r = wqk_d.ap().rearrange("(n p) m -> n p m", p=128)
                wqk_m = wqk_d.ap().rearrange("(n p) m -> p n m", p=128)
                wv_r = wv_d.ap().rearrange("(n p) m -> n p m", p=128)
                # wqk columns host-ordered [q01 | k01 | q23 | k23]: phase A
                # needs only cols 0:256, so those stream per-chunk (same
                # pacing, half the bytes) and the heads-2/3 half follows the
                # critical hsT stream as ONE merged DMA (single HWDGE issue).
                for k in range(KD):
                    nc.sync.dma_start(out=wqk_t[:, k, 0:256], in_=wqk_r[k][:, 0:256])
                    if k < KD - 1:
                        nc.sync.dma_start(out=hsT[k][:], in_=hsT_r[k])
                    else:
                        # split the last chunk: the c0a/c2a slices of the final
                        # k-batch need only tokens 0:1024, so the first scores
                        # can start one half-transfer earlier
                        nc.sync.dma_start(out=hsT[k][:, 0:1024], in_=hsT_r[k][:, 0:1024])
                        nc.sync.dma_start(out=hsT[k][:, 1024:2048], in_=hsT_r[k][:, 1024:2048])
                nc.sync.dma_start(out=wqk_t[:, :, 256:512], in_=wqk_m[:, :, 256:512])
                for k in range(KD):
                    nc.sync.dma_start(out=wv[k][:], in_=wv_r[k])
                nc.sync.dma_start(out=msk_t[:], in_=msk_d.ap().rearrange("(a p) -> p a", p=128))

                PROJ_COL = {0: 0, 2: 128, 1: 256, 3: 384}

                # ---- phase A: QK projection for heads 0/1, k-outer so PE
                # tracks the DMA stream. Only THREE slots: c0a (Q tok 0:1024)
                # plus c2a/c2b (K, all tokens) gate the first attention
                # blocks; c0b (Q tok 1024:2048) is only needed from block 2,
                # so it runs as filler units inside phase B instead.
                with tc.tile_pool(name="pjA", bufs=1, space="PSUM") as pjA:
                    slots = [
                        (pjA.tile([128, 1024], F32, tag=f"pj{i}", name=f"pj{i}"), c, th)
                        for i, (c, th) in enumerate([(0, 0), (2, 0), (2, 1)])
                    ]
                    # copies are emitted right after each slot's final k-chunk
                    # matmul so they overlap the remaining k7 matmuls; ACT is
                    # idle here so it takes the two K copies.
                    def _qkcopy(i):
                        sl, c, th = slots[i]
                        if c == 0 or th == 1:
                            # c0a and c2b on DVE: keeps the ACT queue clear so
                            # the first exps start right after the c2a copy
                            nc.vector.tensor_copy(KT2[0][:, th * 1024 : (th + 1) * 1024] if c else QT2[0][:, th * 1024 : (th + 1) * 1024], sl[:])
                        else:
                            nc.scalar.activation(
                                KT2[0][:, th * 1024 : (th + 1) * 1024], sl[:], AF.Copy
                            )

                    for k in range(KD):
                        for i, (sl, c, th) in enumerate(slots):
                            for t2 in range(2):
                                nc.tensor.matmul(
                                    sl[:, t2 * 512 : (t2 + 1) * 512],
                                    wqk[k][:, PROJ_COL[c] : PROJ_COL[c] + 128],
                                    hsT[k][:, th * 1024 + t2 * 512 : th * 1024 + (t2 + 1) * 512],
                                    start=(k == 0),
                                    stop=(k == KD - 1),
                                )
                            if k == KD - 1:
                                _qkcopy(i)

                # ---- phase B: attention with V proj + h2/h3 QK proj woven in
                with (
                    tc.tile_pool(name="psB", bufs=1, space="PSUM") as ps,
                    tc.tile_pool(name="ptp", bufs=1) as ptp,
                    tc.tile_pool(name="stg", bufs=1) as stg,
                ):
                    # filler units: half-granularity so a single unit does not
                    # starve the ACT exp stream (emitted between dependent
                    # attention stages as PE filler).
                    fstate = {}

                    def vproj_half(t, half):
                        if half == 0:
                            fstate[("v", t)] = ps.tile(
                                [128, 512], F32, tag="aux", bufs=1, name=f"pv{t}"
                            )
                        sl = fstate[("v", t)]
                        for k in range(half * 4, half * 4 + 4):
                            nc.tensor.matmul(
                                sl[:, 0:256],
                                hsT[k][:, t * 128 : (t + 1) * 128],
                                wv[k][:],
                                start=(k == 0),
                                stop=(k == KD - 1),
                            )
                        if half == 1:
                            nc.vector.tensor_copy(
                                Vau[t][:, :, 0:D],
                                sl[:, 0:256].rearrange("p (h d) -> p h d", h=HPC),
                            )
                            nc.gpsimd.tensor_scalar_mul(
                                Vau[t][:], Vau[t][:], msk_t[:, t : t + 1]
                            )
                            del fstate[("v", t)]

                    def cproj_part(c, tq, part, nparts):
                        if part == 0:
                            fstate[(c, tq)] = ps.tile(
                                [128, 512], F32, tag="aux", bufs=1, name=f"pc{c}_{tq}"
                            )
                        sl = fstate[(c, tq)]
                        kpp = KD // nparts
                        for k in range(part * kpp, (part + 1) * kpp):
                            nc.tensor.matmul(
                                sl[:],
                                wqk[k][:, PROJ_COL[c] : PROJ_COL[c] + 128],
                                hsT[k][:, tq * 512 : (tq + 1) * 512],
                                start=(k == 0),
                                stop=(k == KD - 1),
                            )
                        if part == nparts - 1:
                            dst = {0: QT2[0], 1: QT2[1], 3: KT2[1]}[c]
                            nc.vector.tensor_copy(dst[:, tq * 512 : (tq + 1) * 512], sl[:])
                            del fstate[(c, tq)]

                    def cproj_half(c, tq, half):
                        cproj_part(c, tq, half, 2)

                    from collections import deque

                    # c0b (Q heads 0/1, tokens 1024:2048) leads the filler
                    # queue: needed by block 2's scores, and its inputs are
                    # already resident when phase B starts.
                    vfill = deque(
                        [
                            (lambda tq=tq, hf=hf: cproj_half(0, tq, hf))
                            for tq in (2, 3)
                            for hf in (0, 1)
                        ]
                        + [
                            (lambda t=t, hf=hf: vproj_half(t, hf))
                            for t in range(NJT)
                            for hf in (0, 1)
                        ]
                    )
                    cfill = deque(
                        lambda c=c, tq=tq, hf=hf: cproj_half(c, tq, hf)
                        for tq in range(4)
                        for c in (1, 3)
                        for hf in (0, 1)
                    )

                    NCH = IB // 128  # i-chunks per block
                    # j-tile groups per block: 4 triples + 2 pairs so the sc
                    # slot is [128, 1536] (3 banks x 2 bufs) and most exps run
                    # at ap=1536, amortizing the ACT per-instruction init.
                    GROUPS = [(0, 3), (3, 6), (6, 9), (9, 12), (12, 14), (14, 16)]
                    GSTART = [s for s, _ in GROUPS]

                    def _pslice(Ps, jt, ic):
                        g = max(i for i, s in enumerate(GSTART) if s <= jt)
                        loc = jt - GSTART[g]
                        return Ps[g][:, loc * 512 + ic * 128 : loc * 512 + (ic + 1) * 128]

                    def attnv_piece(nc, vob, Ps, h, ic):
                        """One i-chunk accumulation group: 16 consecutive
                        matmuls (interleaved groups within one PSUM bank are
                        illegal; distinct groups in the same bank must be
                        strictly sequential in PE issue order, which ic-outer
                        emission guarantees)."""
                        for jt in range(NJT):
                            nc.tensor.matmul(
                                vob[:, ic, :],
                                _pslice(Ps, jt, ic),
                                Vau[jt][:, h, :],
                                start=(jt == 0),
                                stop=(jt == NJT - 1),
                            )

                    def store_block(nc, blk, vob, h, i0):
                        ot = stg.tile([128, NCH, D], F32, tag="ot", bufs=2, name=f"ot{blk}")
                        for ic in range(NCH):
                            rcp = stg.tile([128, 1], F32R, tag="rcp", bufs=8, name=f"rcp{blk}_{ic}")
                            with nc.allow_low_precision("f32r is bit-identical to f32"):
                                nc.vector.reciprocal(rcp[:], vob[:, ic, D : D + 1].bitcast(F32R))
                            nc.vector.tensor_scalar_mul(
                                ot[:, ic, :], vob[:, ic, 0:D], rcp[:, 0:1].bitcast(F32)
                            )
                        nc.sync.dma_start(
                            out=out_d.ap()[i0 : i0 + IB, h * D : (h + 1) * D].rearrange(
                                "(ic p) d -> p ic d", p=128
                            ),
                            in_=ot[:],
                        )

                    NBLK = HPC * NIB
                    NG = len(GROUPS)
                    DEFER = 3  # attnV runs this many blocks behind its scores
                    pending = []  # (blk, Ps, h, i0) awaiting attnV, oldest first
                    state = {}  # blk -> vob (allocated at burst emission)
                    store_ready = None
                    for blk in range(NBLK):
                        h, ib = blk // NIB, blk % NIB
                        hp, hoff = h // 2, (h % 2) * 64
                        i0 = ib * IB
                        Ps = []
                        for g, (js, je) in enumerate(GROUPS):
                            gw = (je - js) * 512
                            ssl = ps.tile([128, 1536], F32, tag="sc", bufs=2, name=f"sc{blk}_{g}")
                            for jj in range(je - js):
                                jt = js + jj
                                nc.tensor.matmul(
                                    ssl[:, jj * 512 : (jj + 1) * 512],
                                    KT2[hp][hoff : hoff + 64, jt * 128 : (jt + 1) * 128],
                                    QT2[hp][hoff : hoff + 64, i0 : i0 + IB],
                                    start=True,
                                    stop=True,
                                    tile_position=(hoff, 0),
                                )
                            P = ptp.tile([128, 1536], BF16, tag="pt", bufs=NG * (DEFER + 1),
                                         name=f"pt{blk}_{g}")
                            nc.scalar.activation(P[:, 0:gw], ssl[:, 0:gw], AF.Exp,
                                                 bias=0.0, scale=SCALE)
                            Ps.append(P)
                            # store for the block whose pieces finished last
                            # block (shifted to g0 so the piece window starts
                            # at g2, freeing blk0's coldest slots from filler)
                            if g == 0 and store_ready is not None:
                                store_block(nc, *store_ready)
                                store_ready = None
                            # attnV for the block finished DEFER blocks ago, one
                            # i-chunk piece per group slot (keeps PE fed without
                            # head-of-line blocking); vob allocated here so
                            # only one is ever live.
                            if len(pending) >= DEFER and 2 <= g <= NCH + 1:
                                pblk, pPs, ph, pi0 = pending[0]
                                if g == 2:
                                    state[pblk] = ps.tile(
                                        [128, NCH, D + 1], F32, tag="vo", bufs=1, name=f"vo{pblk}"
                                    )
                                attnv_piece(nc, state[pblk], pPs, ph, g - 2)
                                if g == NCH + 1:
                                    store_ready = (pblk, state.pop(pblk), ph, pi0)
                                    pending.pop(0)
                            # filler PE work between dependent stages. All V
                            # halves MUST be emitted before the first attnv
                            # piece that reads Vau (else PE head-of-line
                            # deadlocks on a Vau copy whose matmuls sit behind
                            # it in the queue); blk0's two coldest slots stay
                            # filler-free (PE still at the mid p-state).
                            if blk == 0 and g < 2:
                                pass
                            elif vfill:
                                vfill.popleft()()
                                if vfill:
                                    vfill.popleft()()
                            elif cfill and g in (0, 1, 3):
                                cfill.popleft()()
                        pending.append((blk, Ps, h, i0))
                    if store_ready is not None:
                        store_block(nc, *store_ready)
                        store_ready = None

                    # epilogue: all but the last pending block run as normal
                    # bursts (their exps retired long ago); the last block's
                    # four i-chunk groups go to FOUR different PSUM banks so
                    # their matmuls can interleave jt-by-jt and retire while
                    # the final exps are still streaming.
                    for pblk, pPs, ph, pi0 in pending[:-1]:
                        vob = ps.tile([128, NCH, D + 1], F32, tag="vo", bufs=1, name=f"vo{pblk}")
                        for ic in range(NCH):
                            attnv_piece(nc, vob, pPs, ph, ic)
                        store_block(nc, pblk, vob, ph, pi0)
                    lblk, lPs, lh, li0 = pending[-1]
                    voA = ps.tile([128, NCH, D + 1], F32, tag="vo", bufs=1, name="voLa")
                    auxA = ps.tile([128, 512], F32, tag="aux", bufs=1, name="auxLa")
                    # one sc slot provides two more banks (cols 0:512 and
                    # 512:1024 live in different banks, so their accumulation
                    # groups may interleave)
                    scX = ps.tile([128, 1536], F32, tag="sc", bufs=2, name="scLx")
                    lvo = [
                        voA[:, 0, :],
                        auxA[:, 0 : D + 1],
                        scX[:, 0 : D + 1],
                        scX[:, 512 : 512 + D + 1],
                    ]
                    for jt in range(NJT):
                        for ic in range(NCH):
                            nc.tensor.matmul(
                                lvo[ic],
                                _pslice(lPs, jt, ic),
                                Vau[jt][:, lh, :],
                                start=(jt == 0),
                                stop=(jt == NJT - 1),
                            )
                    ot = stg.tile([128, NCH, D], F32, tag="ot", bufs=2, name="otL")
                    for ic in range(NCH):
                        rcp = stg.tile([128, 1], F32R, tag="rcp", bufs=8, name=f"rcpL{ic}")
                        with nc.allow_low_precision("f32r is bit-identical to f32"):
                            nc.vector.reciprocal(rcp[:], lvo[ic][:, D : D + 1].bitcast(F32R))
                        nc.vector.tensor_scalar_mul(
                            ot[:, ic, :], lvo[ic][:, 0:D], rcp[:, 0:1].bitcast(F32)
                        )
                    nc.sync.dma_start(
                        out=out_d.ap()[li0 : li0 + IB, lh * D : (lh + 1) * D].rearrange(
                            "(ic p) d -> p ic d", p=128
                        ),
                        in_=ot[:],
                    )
    if not nc.is_finalized():
        nc.finalize()
    _CACHE["nc"] = nc
    return nc


def kernel(hidden_states, attention_mask, W_qkv):
    import ml_dtypes

    hs = np.asarray(hidden_states, dtype=np.float32)  # [2, 2048, 1024]
    am = np.asarray(attention_mask)  # [2, 2048]
    W = np.asarray(W_qkv, dtype=np.float32)  # [16, 1024, 192]

    nc = _build()
    in_maps = []
    for core in range(NCORES):
        b, hg = core // 4, core % 4
        Wc = W[hg * 4 : hg * 4 + 4]  # [4, 1024, 192]
        q = [Wc[h, :, 0:64] for h in range(4)]
        k = [Wc[h, :, 64:128] for h in range(4)]
        v = [Wc[h, :, 128:192] for h in range(4)]
        in_maps.append(
            {
                "hsT": np.ascontiguousarray(hs[b].T).astype(ml_dtypes.bfloat16),
                "wqk": np.ascontiguousarray(
                    np.concatenate([q[0], q[1], k[0], k[1], q[2], q[3], k[2], k[3]], axis=1)
                ).astype(ml_dtypes.bfloat16),
                "wv": np.ascontiguousarray(np.concatenate(v, axis=1)).astype(ml_dtypes.bfloat16),
                "msk": (am[b] != 0).astype(np.float32),
            }
        )
    res = run_bass_kernel_spmd(nc, in_maps, list(range(NCORES)))
    if res.exec_time_ns is not None:
        print(f"HW exec time: {res.exec_time_ns} ns")
    if res.mean_exec_time_ns is not None:
        print(f"HW exec time (mean across cores): {res.mean_exec_time_ns} ns")
    out = np.empty((2, NT, HID), dtype=np.float32)
    for core in range(NCORES):
        b, hg = core // 4, core % 4
        out[b, :, hg * 256 : (hg + 1) * 256] = res.results[core]["out"]
    return out


def predicted_exec_ns():
    """Device-occupancy estimate for one core (all 8 run the same program
    in parallel)."""
    nc = _build()
    from concourse.timeline_sim import TimelineSim
    return float(TimelineSim(nc, trace=False).simulate())
